# revision 8
# baseline (speedup 1.0000x reference)
"""Trainium2 Bass kernel for AttentionalPlanarRemapping.

out[n,c,h,w] = sum_d softmax(atts[n,c,:])[d] * images[n,d,h,w]

Per-sample: W = softmax(atts[n]) [C,C]; out[n] = W @ images[n].reshape(C, H*W).

Sharding: data-parallel over N across 8 cores (4 samples per core).

Host preprocessing inside kernel(): atts is passed TRANSPOSED per sample
(attsT[n] = atts[n].T, layout [d, c]) and cast to fp16, so no on-device
transposition is needed and the atts DMA is halved: attsT loads with the
contraction dim d on partitions, which is exactly the matmul lhsT layout.

images are uploaded as fp16 and the output is stored as fp16 (values only --
the returned array is float32): fp16's 11-bit mantissa matches the PE's
full-rate matmul path while halving DMA traffic. |atts| < 6 so exp needs
no max-subtraction and exp(atts) < 448 fits fp16 comfortably.

Per-core plan:
  - ALL input DMAs are issued up front on the sync (HWDGE) queue in
    priority order; every sample has its own SBUF tiles (no recycling), so
    loads stream at full rate and are never blocked by compute.
    Sample 0's tensors are loaded in fine chunks (atts halves + image
    quarters) and exp(0) runs in per-kd chunks so the first matmul can
    start ~2us in instead of waiting for whole-sample loads.
  - Main matmuls run on UNNORMALIZED E = exp(attsT): for each output block
    kc (128 rows of c) and each psum half ht, accumulate over kd:
        ps[ht][c128, 512] += E[kd-blk, kc-blk].T @ X[kd-blk, ht-half]
    Each [128,512] f32 psum tile is exactly one PSUM bank; 6 bufs rotate.
  - Softmax denominators ride along as tiny N=2 matmuls on the SAME
    loaded weights: sm[kc][c128, 2] += E[kd-blk, kc-blk].T @ ones[128,2].
    This lands s[c] directly in per-partition layout (no fp32
    redistribution matmuls, no extra DVE copies). Two single-bank sm
    tiles alternate between kc's so the DVE reciprocal read never
    collides with PE writes to the same bank.
  - r = 1/s via DVE reciprocal; evictions scale by r while casting to
    fp16: ht=0 on ACT (scalar.mul), ht=1 on DVE (tensor_scalar_mul).
  - exp(n+1) is emitted between kc=1 and kc=2 of compute(n) so the ACT
    queue never head-of-line blocks evictions on a pending DMA wait.
  - Stores stream per-kc (0.25 MB) on the gpsimd (SWDGE) queue, which is
    otherwise idle, so stores never contend with load issuance.
"""

import numpy as np
from contextlib import ExitStack

import concourse.bass as bass
import concourse.mybir as mybir
import concourse.tile as tile
from concourse import bacc
from concourse.bass_utils import run_bass_kernel_spmd

N, C, H, W = 32, 512, 32, 32
HW = H * W                      # 1024
NCORES = 8
NPC = N // NCORES               # 4 samples per core
P = 128
KC = C // P                     # 4 chunks over output channel c
KD = C // P                     # 4 chunks over contraction d
NT = 512                        # matmul moving free dim (one PSUM bank of f32)
NHT = HW // NT                  # 2

F32 = mybir.dt.float32
F16 = mybir.dt.float16
AF = mybir.ActivationFunctionType


def build_nc():
    nc = bacc.Bacc("TRN2", target_bir_lowering=False, debug=False)

    images = nc.dram_tensor("images", [NPC, C, HW], F16, kind="ExternalInput").ap()
    attsT = nc.dram_tensor("attsT", [NPC, C, C], F16, kind="ExternalInput").ap()
    out = nc.dram_tensor("out", [NPC, C, HW], F16, kind="ExternalOutput").ap()

    with ExitStack() as ctx:
        tc = ctx.enter_context(tile.TileContext(nc))

        const_pool = ctx.enter_context(tc.tile_pool(name="const", bufs=1))
        ones_f32 = const_pool.tile([P, 2], F32)
        ones = const_pool.tile([P, 2], F16)
        warm_x = const_pool.tile([P, NT], F16)

        a_pool = ctx.enter_context(tc.tile_pool(name="a", bufs=1))
        e_pool = ctx.enter_context(tc.tile_pool(name="e", bufs=1))
        x_pool = ctx.enter_context(tc.tile_pool(name="x", bufs=1))
        o_pool = ctx.enter_context(tc.tile_pool(name="o", bufs=4))
        r_pool = ctx.enter_context(tc.tile_pool(name="r", bufs=2))
        mm_psum = ctx.enter_context(tc.tile_pool(name="mmp", bufs=6, space="PSUM"))
        sm_psum = ctx.enter_context(tc.tile_pool(name="smp", bufs=2, space="PSUM"))

        # ---- constants ----
        nc.gpsimd.memset(ones_f32[:], 1.0)
        nc.vector.tensor_copy(ones[:], ones_f32[:])
        nc.gpsimd.memset(warm_x[:], 0.0)

        # ---- all input DMAs up front (sync/HWDGE queue, priority order) ----
        a_tiles = []
        x_tiles = []
        for n in range(NPC):
            a_tiles.append(a_pool.tile([P, KD, C], F16, name=f"a{n}", tag=f"a{n}"))
            x_tiles.append(x_pool.tile([P, KD, HW], F16, name=f"x{n}", tag=f"x{n}"))

        def load_a_quarter(n, kd):
            nc.sync.dma_start(
                a_tiles[n][:, kd : kd + 1],
                attsT[n][kd * P : (kd + 1) * P].rearrange("(kd p) c -> p kd c", p=P),
            )

        def load_a_halves(n):
            for h in range(2):
                nc.sync.dma_start(
                    a_tiles[n][:, h * 2 : (h + 1) * 2],
                    attsT[n][h * 256 : (h + 1) * 256].rearrange(
                        "(kd p) c -> p kd c", p=P
                    ),
                )

        def load_x_quarter(n, kd):
            nc.sync.dma_start(
                x_tiles[n][:, kd : kd + 1],
                images[n][kd * P : (kd + 1) * P].rearrange("(kd p) f -> p kd f", p=P),
            )

        def load_x_halves(n):
            for h in range(2):
                nc.sync.dma_start(
                    x_tiles[n][:, h * 2 : (h + 1) * 2],
                    images[n][h * 256 : (h + 1) * 256].rearrange(
                        "(kd p) f -> p kd f", p=P
                    ),
                )

        # sample 0 fine-grained and kd-interleaved so exp/matmuls start ASAP
        load_a_quarter(0, 0)
        load_x_quarter(0, 0)
        load_a_quarter(0, 1)
        load_x_quarter(0, 1)
        load_a_quarter(0, 2)
        load_x_quarter(0, 2)
        load_a_quarter(0, 3)
        load_x_quarter(0, 3)
        load_a_halves(1)
        load_x_halves(1)
        load_a_halves(2)
        load_x_halves(2)
        load_a_halves(3)
        load_x_halves(3)

        # ---- exp ----
        e_tiles = [e_pool.tile([P, KD, C], F16, name=f"e{n}", tag=f"e{n}") for n in range(NPC)]

        def emit_exp_chunk(n, kd):
            """exp of one kd chunk of sample n (ACT, ~0.6us each)."""
            nc.scalar.activation(
                e_tiles[n][:, kd : kd + 1],
                a_tiles[n][:, kd : kd + 1],
                AF.Exp,
                bias=0.0,
                scale=1.0,
            )

        for kd in range(KD):
            emit_exp_chunk(0, kd)

        # ---- PE warm-up: ~16 dummy matmuls so HAM reaches 8/8 before the
        # real stream starts (first ~3.4us of PE activity runs at 1.2 GHz).
        # Output goes to a scratch slot of the mm psum pool (slot is released
        # long before the pool wraps around to it).
        warm_ps = mm_psum.tile([P, NT], F32, name="warm_ps", tag="ps", space="PSUM")
        for _ in range(16):
            nc.tensor.matmul(
                warm_ps[0:2, :], lhsT=ones[:], rhs=warm_x[:], start=True, stop=True
            )

        def compute(n):
            e_t = e_tiles[n]
            x_t = x_tiles[n]
            r_sb = r_pool.tile([P, KC], F32, name=f"r{n}", tag="r")
            for kc in range(KC):
                ps = [
                    mm_psum.tile(
                        [P, NT], F32, name=f"ps{n}_{kc}_{ht}", tag="ps", space="PSUM"
                    )
                    for ht in range(NHT)
                ]
                sm = sm_psum.tile([P, 2], F32, name=f"sm{n}_{kc}", tag="sm",
                                  space="PSUM")
                for kd in range(KD):
                    lhs = e_t[:, kd, kc * P : (kc + 1) * P]
                    for ht in range(NHT):
                        nc.tensor.matmul(
                            ps[ht][:],
                            lhsT=lhs,
                            rhs=x_t[:, kd, ht * NT : (ht + 1) * NT],
                            start=(kd == 0),
                            stop=(kd == KD - 1),
                        )
                    nc.tensor.matmul(
                        sm[:],
                        lhsT=lhs,
                        rhs=ones[:],
                        start=(kd == 0),
                        stop=(kd == KD - 1),
                    )
                r_ap = r_sb[:, kc : kc + 1]
                nc.vector.reciprocal(r_ap, sm[:, 0:1])
                o_t = o_pool.tile([P, HW], F16, name=f"o{n}_{kc}", tag="o")
                nc.scalar.mul(o_t[:, 0:NT], ps[0][:], r_ap)
                nc.vector.tensor_scalar_mul(o_t[:, NT:], ps[1][:], r_ap)
                last = n == NPC - 1 and kc == KC - 1
                if last:
                    # split the final store so its second half (HWDGE, low
                    # start latency) begins as soon as the DVE eviction ends
                    nc.scalar.dma_start(
                        out[n][kc * P : (kc + 1) * P, 0:NT], o_t[:, 0:NT]
                    )
                    nc.scalar.dma_start(
                        out[n][kc * P : (kc + 1) * P, NT:], o_t[:, NT:]
                    )
                else:
                    nc.gpsimd.dma_start(out[n][kc * P : (kc + 1) * P], o_t[:])
                if n + 1 < NPC:
                    emit_exp_chunk(n + 1, kc)

        for n in range(NPC):
            compute(n)

    nc.compile()
    return nc


_NC_CACHE = None


def _get_nc():
    global _NC_CACHE
    if _NC_CACHE is None:
        _NC_CACHE = build_nc()
    return _NC_CACHE


def run(in_maps, **kwargs):
    """Run the SPMD kernel on cores 0..7. in_maps: one dict per core."""
    nc = _get_nc()
    return run_bass_kernel_spmd(nc, in_maps, core_ids=list(range(NCORES)), **kwargs)


def make_in_maps(images: np.ndarray, atts: np.ndarray):
    images = np.ascontiguousarray(
        np.asarray(images, dtype=np.float32).astype(np.float16)
    )
    atts = np.asarray(atts, dtype=np.float32)
    assert images.shape == (N, C, H, W), images.shape
    assert atts.shape == (N, C, C), atts.shape
    img_s = images.reshape(NCORES, NPC, C, HW)
    # per-sample transpose: attsT[n] = atts[n].T  (layout [d, c]), fp16
    attsT = np.ascontiguousarray(atts.transpose(0, 2, 1).astype(np.float16)).reshape(
        NCORES, NPC, C, C
    )
    return [
        {"images": np.ascontiguousarray(img_s[i]), "attsT": attsT[i]}
        for i in range(NCORES)
    ]


def kernel(images: np.ndarray, atts: np.ndarray) -> np.ndarray:
    in_maps = make_in_maps(images, atts)
    res = run(in_maps)
    outs = [res.results[i]["out"] for i in range(NCORES)]
    full = np.concatenate(outs, axis=0).reshape(N, C, H, W)
    return full.astype(np.float32)


# revision 13
# speedup vs baseline: 1.1966x; 1.1966x over previous
"""Trainium2 Bass kernel for AttentionalPlanarRemapping.

out[n,c,h,w] = sum_d softmax(atts[n,c,:])[d] * images[n,d,h,w]

Per-sample: W = softmax(atts[n]) [C,C]; out[n] = W @ images[n].reshape(C, H*W).

Sharding: data-parallel over N across 8 cores (4 samples per core).

Host preprocessing inside kernel(): atts is passed TRANSPOSED per sample
(attsT[n] = atts[n].T, layout [d, c]) and cast to fp16, so no on-device
transposition is needed and the atts DMA is halved: attsT loads with the
contraction dim d on partitions, which is exactly the matmul lhsT layout.

images are uploaded as fp16 and the output is stored as fp16 (values only --
the returned array is float32): fp16's 11-bit mantissa matches the PE's
full-rate matmul path while halving DMA traffic. |atts| < 6 so exp needs
no max-subtraction and exp(atts) < 448 fits fp16 comfortably.

Per-core plan:
  - ALL input DMAs are issued up front on the sync (HWDGE) queue in
    priority order; every sample has its own SBUF tiles (no recycling), so
    loads stream at full rate and are never blocked by compute.
    Sample 0's tensors are loaded in fine chunks (atts halves + image
    quarters) and exp(0) runs in per-kd chunks so the first matmul can
    start ~2us in instead of waiting for whole-sample loads.
  - Main matmuls run on UNNORMALIZED E = exp(attsT): for each output block
    kc (128 rows of c) and each psum half ht, accumulate over kd:
        ps[ht][c128, 512] += E[kd-blk, kc-blk].T @ X[kd-blk, ht-half]
    Each [128,512] f32 psum tile is exactly one PSUM bank; 6 bufs rotate.
  - Softmax denominators ride along as tiny N=2 matmuls on the SAME
    loaded weights: sm[kc][c128, 2] += E[kd-blk, kc-blk].T @ ones[128,2].
    This lands s[c] directly in per-partition layout (no fp32
    redistribution matmuls, no extra DVE copies). Two single-bank sm
    tiles alternate between kc's so the DVE reciprocal read never
    collides with PE writes to the same bank.
  - r = 1/s via DVE reciprocal; evictions scale by r while casting to
    fp16: ht=0 on ACT (scalar.mul), ht=1 on DVE (tensor_scalar_mul).
  - exp(n+1) is emitted between kc=1 and kc=2 of compute(n) so the ACT
    queue never head-of-line blocks evictions on a pending DMA wait.
  - Stores stream per-kc (0.25 MB) on the gpsimd (SWDGE) queue, which is
    otherwise idle, so stores never contend with load issuance.
"""

import numpy as np
from contextlib import ExitStack

import concourse.bass as bass
import concourse.mybir as mybir
import concourse.tile as tile
from concourse import bacc
from concourse.bass_utils import run_bass_kernel_spmd

N, C, H, W = 32, 512, 32, 32
HW = H * W                      # 1024
NCORES = 8
NPC = N // NCORES               # 4 samples per core
P = 128
KC = C // P                     # 4 chunks over output channel c
KD = C // P                     # 4 chunks over contraction d
NT = 512                        # matmul moving free dim (one PSUM bank of f32)
NHT = HW // NT                  # 2

F32 = mybir.dt.float32
F16 = mybir.dt.float16
AF = mybir.ActivationFunctionType


def build_nc():
    nc = bacc.Bacc("TRN2", target_bir_lowering=False, debug=False)

    images = nc.dram_tensor("images", [NPC, C, HW], F16, kind="ExternalInput").ap()
    attsT = nc.dram_tensor("attsT", [NPC, C, C], F16, kind="ExternalInput").ap()
    out = nc.dram_tensor("out", [NPC, C, HW], F16, kind="ExternalOutput").ap()

    with ExitStack() as ctx:
        tc = ctx.enter_context(tile.TileContext(nc))

        const_pool = ctx.enter_context(tc.tile_pool(name="const", bufs=1))
        ones_f32 = const_pool.tile([P, 2], F32)
        ones = const_pool.tile([P, 2], F16)
        warm_x = const_pool.tile([P, NT], F16)

        a_pool = ctx.enter_context(tc.tile_pool(name="a", bufs=1))
        e_pool = ctx.enter_context(tc.tile_pool(name="e", bufs=1))
        x_pool = ctx.enter_context(tc.tile_pool(name="x", bufs=1))
        o_pool = ctx.enter_context(tc.tile_pool(name="o", bufs=4))
        r_pool = ctx.enter_context(tc.tile_pool(name="r", bufs=2))
        mm_psum = ctx.enter_context(tc.tile_pool(name="mmp", bufs=6, space="PSUM"))
        sm_psum = ctx.enter_context(tc.tile_pool(name="smp", bufs=2, space="PSUM"))

        # ---- constants ----
        nc.gpsimd.memset(warm_x[:], 0.0)
        nc.gpsimd.memset(ones_f32[:], 1.0)
        nc.vector.tensor_copy(ones[:], ones_f32[:])

        # ---- all input DMAs up front (sync/HWDGE queue, priority order) ----
        a_tiles = []
        x_tiles = []
        for n in range(NPC):
            a_tiles.append(a_pool.tile([P, KD, C], F16, name=f"a{n}", tag=f"a{n}"))
            x_tiles.append(x_pool.tile([P, KD, HW], F16, name=f"x{n}", tag=f"x{n}"))

        def load_a_quarter(n, kd):
            nc.sync.dma_start(
                a_tiles[n][:, kd : kd + 1],
                attsT[n][kd * P : (kd + 1) * P].rearrange("(kd p) c -> p kd c", p=P),
            )

        def load_a_halves(n):
            for h in range(2):
                nc.sync.dma_start(
                    a_tiles[n][:, h * 2 : (h + 1) * 2],
                    attsT[n][h * 256 : (h + 1) * 256].rearrange(
                        "(kd p) c -> p kd c", p=P
                    ),
                )

        def load_x_quarter(n, kd):
            nc.sync.dma_start(
                x_tiles[n][:, kd : kd + 1],
                images[n][kd * P : (kd + 1) * P].rearrange("(kd p) f -> p kd f", p=P),
            )

        def load_x_halves(n):
            for h in range(2):
                nc.sync.dma_start(
                    x_tiles[n][:, h * 2 : (h + 1) * 2],
                    images[n][h * 256 : (h + 1) * 256].rearrange(
                        "(kd p) f -> p kd f", p=P
                    ),
                )

        # sample 0 fine-grained and kd-interleaved so exp/matmuls start ASAP:
        # a0's first kd chunk alone (gates the first exp), then the rest of
        # a0 in one efficient transfer, x0 in full-rate 2KB-segment quarters.
        load_a_quarter(0, 0)
        load_x_quarter(0, 0)
        nc.sync.dma_start(
            a_tiles[0][:, 1:KD],
            attsT[0][P:].rearrange("(kd p) c -> p kd c", p=P),
        )
        load_x_quarter(0, 1)
        load_x_quarter(0, 2)
        load_x_quarter(0, 3)
        load_a_halves(1)
        load_x_halves(1)
        load_a_halves(2)
        load_x_halves(2)
        load_a_halves(3)
        load_x_halves(3)

        # ---- exp ----
        e_tiles = [e_pool.tile([P, KD, C], F16, name=f"e{n}", tag=f"e{n}") for n in range(NPC)]

        def emit_exp_chunk(n, kd):
            """exp of one kd chunk of sample n (ACT, ~0.6us each)."""
            nc.scalar.activation(
                e_tiles[n][:, kd : kd + 1],
                a_tiles[n][:, kd : kd + 1],
                AF.Exp,
                bias=0.0,
                scale=1.0,
            )

        for kd in range(KD):
            emit_exp_chunk(0, kd)

        # ---- PE warm-up: dummy matmuls so HAM reaches 8/8 before the real
        # stream starts (first ~3.4us of PE activity runs at 1.2 GHz). Gated
        # only on the warm_x memset; sized to end as the first real matmul's
        # inputs land (~11us: DMA + completion receipt + exp). Output goes to
        # a scratch slot of the mm psum pool (released long before the pool
        # wraps around to it).
        warm_ps = mm_psum.tile([P, NT], F32, name="warm_ps", tag="ps", space="PSUM")
        for _ in range(8):
            nc.tensor.matmul(
                warm_ps[0:2, :],
                lhsT=warm_x[:, 0:2],
                rhs=warm_x[:],
                start=True,
                stop=True,
            )

        def compute(n):
            e_t = e_tiles[n]
            x_t = x_tiles[n]
            r_sb = r_pool.tile([P, KC], F32, name=f"r{n}", tag="r")
            for kc in range(KC):
                ps = [
                    mm_psum.tile(
                        [P, NT], F32, name=f"ps{n}_{kc}_{ht}", tag="ps", space="PSUM"
                    )
                    for ht in range(NHT)
                ]
                sm = sm_psum.tile([P, 2], F32, name=f"sm{n}_{kc}", tag="sm",
                                  space="PSUM")
                for kd in range(KD):
                    lhs = e_t[:, kd, kc * P : (kc + 1) * P]
                    # tiny sum-matmul first: its kd=3 stop gates the
                    # reciprocal, so retiring it before the two 512-col
                    # matmuls shortens the eviction critical path
                    nc.tensor.matmul(
                        sm[:],
                        lhsT=lhs,
                        rhs=ones[:],
                        start=(kd == 0),
                        stop=(kd == KD - 1),
                    )
                    for ht in range(NHT):
                        nc.tensor.matmul(
                            ps[ht][:],
                            lhsT=lhs,
                            rhs=x_t[:, kd, ht * NT : (ht + 1) * NT],
                            start=(kd == 0),
                            stop=(kd == KD - 1),
                        )
                r_ap = r_sb[:, kc : kc + 1]
                nc.vector.reciprocal(r_ap, sm[:, 0:1])
                o_t = o_pool.tile([P, HW], F16, name=f"o{n}_{kc}", tag="o")
                nc.scalar.mul(o_t[:, 0:NT], ps[0][:], r_ap)
                nc.vector.tensor_scalar_mul(o_t[:, NT:], ps[1][:], r_ap)
                last = n == NPC - 1 and kc == KC - 1
                if last:
                    # split the final store across two queues so the halves
                    # transfer in parallel as their evictions finish
                    nc.gpsimd.dma_start(
                        out[n][kc * P : (kc + 1) * P, 0:NT], o_t[:, 0:NT]
                    )
                    nc.scalar.dma_start(
                        out[n][kc * P : (kc + 1) * P, NT:], o_t[:, NT:]
                    )
                else:
                    nc.gpsimd.dma_start(out[n][kc * P : (kc + 1) * P], o_t[:])
                if n + 1 < NPC:
                    emit_exp_chunk(n + 1, kc)

        for n in range(NPC):
            compute(n)

    nc.compile()
    return nc


_NC_CACHE = None


def _get_nc():
    global _NC_CACHE
    if _NC_CACHE is None:
        _NC_CACHE = build_nc()
    return _NC_CACHE


def run(in_maps, **kwargs):
    """Run the SPMD kernel on cores 0..7. in_maps: one dict per core."""
    nc = _get_nc()
    return run_bass_kernel_spmd(nc, in_maps, core_ids=list(range(NCORES)), **kwargs)


def make_in_maps(images: np.ndarray, atts: np.ndarray):
    images = np.ascontiguousarray(
        np.asarray(images, dtype=np.float32).astype(np.float16)
    )
    atts = np.asarray(atts, dtype=np.float32)
    assert images.shape == (N, C, H, W), images.shape
    assert atts.shape == (N, C, C), atts.shape
    img_s = images.reshape(NCORES, NPC, C, HW)
    # per-sample transpose: attsT[n] = atts[n].T  (layout [d, c]), fp16
    attsT = np.ascontiguousarray(atts.transpose(0, 2, 1).astype(np.float16)).reshape(
        NCORES, NPC, C, C
    )
    return [
        {"images": np.ascontiguousarray(img_s[i]), "attsT": attsT[i]}
        for i in range(NCORES)
    ]


def kernel(images: np.ndarray, atts: np.ndarray) -> np.ndarray:
    in_maps = make_in_maps(images, atts)
    res = run(in_maps)
    outs = [res.results[i]["out"] for i in range(NCORES)]
    full = np.concatenate(outs, axis=0).reshape(N, C, H, W)
    return full.astype(np.float32)
